# revision 1
# baseline (speedup 1.0000x reference)
"""Trainium2 Bass kernel for BondUpdate GNN message passing.

Computes, for each bond e:
    x = concat(sites[idx1[e]], sites[idx2[e]], bonds[e], states[g2b[e]])  # [896]
    out[e] = relu(relu(relu(x @ W1 + b1) @ W2 + b2) @ W3 + b3)           # [256]

Strategy (v2): the 20000 sites are referenced ~400k times via idx1/idx2, and
512 graph states ~200k times, so the W1 blocks that multiply site/state
features are precomputed ONCE per core into DRAM projection tables:
    A1 = sites @ W1[0:256]      (only the local idx1 range, see below)
    A2 = sites @ W1[256:512]    (full table)
    Ast = states @ W1[768:896]  (full table)
Per bond, layer 1 then reduces to a 256-wide matmul on the bond features plus
a gather+sum of three projection rows, transposed into feature-major via the
PE. This cuts per-tile PE cycles from 72k to 53k vs computing the full
896-wide layer 1 on gathered raw features.

To amortize the A1 table, bonds are sharded across the 8 cores SORTED by
idx1: each core's 25000 bonds reference a ~2500-site contiguous idx1 range,
so A1 is only computed for that local range (A2 still needs the full site
table since idx2 remains scattered). The host un-permutes the output rows.

Activations stay transposed in SBUF (features on partitions, bonds on free
dim) so the three matmul layers chain without intermediate transposes.
Matmul operands are bf16 (PSUM accumulation fp32, biases+relu applied in
fp32, final output stage f32r so values are not re-rounded).
"""
import sys

if "/opt/trn_rl_repo" not in sys.path:
    sys.path.insert(0, "/opt/trn_rl_repo")

import ml_dtypes
import numpy as np

import concourse.bass as bass
import concourse.mybir as mybir
import concourse.tile as tile
from concourse.bass_utils import run_bass_kernel_spmd
from concourse.masks import make_identity
from concourse.vector_clock import ScopedClock

F32 = mybir.dt.float32
F32R = mybir.dt.float32r
BF16 = mybir.dt.bfloat16
I32 = mybir.dt.int32

P = 128            # partitions
T = 512            # bonds per tile
SUB = T // P       # 128-bond subtiles per tile

N_SITES = 20000
N_SITES_PAD = 20480            # 40 * 512: A2 table rows / precompute span
N_GRAPHS = 512
SITE_LEN = 256
BOND_LEN = 256
STATE_LEN = 128
H1 = 1024
H2 = 1024
OUT_DIM = 256

KCB, MC1 = BOND_LEN // P, H1 // P  # 2, 8   (bond block of W1)
KC2, MC2 = H1 // P, H2 // P        # 8, 8
KC3, MC3 = H2 // P, OUT_DIM // P   # 8, 2

N_CORES = 8
N_BONDS = 200000
E_SHARD = N_BONDS // N_CORES       # 25000 bonds per core (sorted by idx1)
TILES_PER_CORE = 49                # 49*512 = 25088 >= 25000
E_CORE = TILES_PER_CORE * T


EVSEM_WAIT_CAP = 2  # InstEventSemaphore holds 2 waits; every other inst 1


def _legalize_waits(nc: bass.Bass):
    """Spill sync waits beyond the per-instruction capacity onto standalone
    InstEventSemaphore instructions inserted just before the offender.
    Walrus here rejects instructions with more waits than the ISA slots."""
    n_spilled = 0
    for f in nc.m.functions:
        for bb in f.blocks:
            il = bb.instructions
            i = 0
            while i < len(il):
                inst = il[i]
                si = inst.sync_info
                waits = list(si.on_wait) if si and si.on_wait else []
                cap = (
                    EVSEM_WAIT_CAP
                    if isinstance(inst, mybir.InstEventSemaphore)
                    else 1
                )
                if len(waits) > cap:
                    keep = waits[-cap:]
                    spill = waits[:-cap]
                    si.on_wait = keep
                    evs = []
                    for j in range(0, len(spill), EVSEM_WAIT_CAP):
                        ev = mybir.InstEventSemaphore(
                            name=nc.get_next_instruction_name(),
                            ins=[],
                            outs=[],
                            sync_info=mybir.SyncInfo(
                                on_wait=spill[j:j + EVSEM_WAIT_CAP],
                                on_update=[],
                            ),
                        )
                        ev.engine = inst.engine
                        nc.register_instruction(ev)
                        evs.append(ev)
                    il[i:i] = evs
                    i += len(evs)
                    n_spilled += len(spill)
                i += 1
    return n_spilled


class SplitDrainTileContext(tile.TileContext):
    """TileContext whose kernel-tail drain also respects the wait cap."""

    def _drain_and_barrier(self, tick_clock, wait_clock):
        nc = self.nc
        drain_inst = nc.sync.drain()
        wait_clock.add_sem_waits(
            drain_inst.ins, ScopedClock({None: tick_clock.global_clock})
        )
        si = drain_inst.ins.sync_info
        waits = list(si.on_wait or [])
        if len(waits) > 1:
            si.on_wait = []
            id2sem = {s.num: s for s in self.sems.allocated().values()}
            for w in waits:
                assert w.wait_mode == "sem-ge-imm", w
                nc.sync.wait_ge(id2sem[w.id], w.wait_value)
        nc.all_engine_barrier()
        assert self.sems is not None
        popped = nc._tile_sem_poison_stack.pop()
        assert popped is self._sem_poison
        nc.clear_and_free_semaphores(list(self.sems.allocated().values()))
        nc.all_engine_barrier()

    def strict_barrier_capped(self):
        """strict_bb_all_engine_barrier, then strip the (possibly hundreds
        of) accumulated backward waits down via the wait legalizer later."""
        self.strict_bb_all_engine_barrier()


def build_bass(n_tiles: int, nloc: int, nch2g: int) -> bass.Bass:
    """Per-core Bass program: n_tiles*T bonds, nloc*128 local A1 rows,
    nch2g*512 deduped A2 rows (sites referenced by this core's idx2)."""
    nc = bass.Bass("TRN2", target_bir_lowering=False, debug=False, num_devices=1)
    E = n_tiles * T
    LSITE = nloc * P
    LS2 = nch2g * 4 * P

    # --- external inputs
    sitesT2 = nc.dram_tensor("sitesT2", [SITE_LEN, LS2], BF16, kind="ExternalInput")
    sitesTloc = nc.dram_tensor("sitesTloc", [SITE_LEN, LSITE], BF16, kind="ExternalInput")
    statesT = nc.dram_tensor("statesT", [STATE_LEN, N_GRAPHS], BF16, kind="ExternalInput")
    bondsT = nc.dram_tensor("bondsT", [BOND_LEN, E], BF16, kind="ExternalInput")
    # indices pre-wrapped on host to [P, n_tiles*SUB]: idx[p, t*SUB+j] = raw[t*T + j*P + p]
    idx1 = nc.dram_tensor("idx1", [P, n_tiles * SUB], I32, kind="ExternalInput")
    idx2 = nc.dram_tensor("idx2", [P, n_tiles * SUB], I32, kind="ExternalInput")
    g2b = nc.dram_tensor("g2b", [P, n_tiles * SUB], I32, kind="ExternalInput")
    # W1 site block rows 0:512 as [p, k, h] = W1[k*128+p, h], k=0..3
    w1s = nc.dram_tensor("w1s", [P, 4, H1], F32, kind="ExternalInput")
    # W1 state block rows 768:896: [p, h] = W1[768+p, h]
    w1st = nc.dram_tensor("w1st", [P, H1], F32, kind="ExternalInput")
    # W1 bond block rows 512:768 chunked: w1c[p, (k*MC1+m)*P+j] = W1[512+k*P+p, m*P+j]
    w1c = nc.dram_tensor("w1c", [P, KCB * MC1 * P], F32, kind="ExternalInput")
    w2c = nc.dram_tensor("w2c", [P, KC2 * MC2 * P], F32, kind="ExternalInput")
    w3c = nc.dram_tensor("w3c", [P, KC3 * MC3 * P], F32, kind="ExternalInput")
    # biases pre-wrapped: bXc[p, m] = bX[m*P+p]
    b1c = nc.dram_tensor("b1c", [P, MC1], F32, kind="ExternalInput")
    b2c = nc.dram_tensor("b2c", [P, MC2], F32, kind="ExternalInput")
    b3c = nc.dram_tensor("b3c", [P, MC3], F32, kind="ExternalInput")
    outT = nc.dram_tensor("outT", [OUT_DIM, E], F32, kind="ExternalOutput")

    # --- internal DRAM projection tables (bf16 rows, gathered per bond)
    A1d = nc.dram_tensor("A1d", [LSITE, H1], BF16, kind="Internal")
    A2d = nc.dram_tensor("A2d", [LS2, H1], BF16, kind="Internal")
    Astd = nc.dram_tensor("Astd", [N_GRAPHS, H1], BF16, kind="Internal")

    with SplitDrainTileContext(nc) as tc:
        with (
            tc.tile_pool(name="const", bufs=1) as constp,
            tc.tile_pool(name="wts", bufs=1) as wp,
            tc.tile_pool(name="idx", bufs=1) as idxp,
            tc.tile_pool(name="gath1", bufs=4) as g1p,
            tc.tile_pool(name="gath2", bufs=2) as g2p,
            tc.tile_pool(name="ssum", bufs=2) as ssump,
            tc.tile_pool(name="s01p", bufs=1) as s01p,
            tc.tile_pool(name="xT", bufs=3) as xp,
            tc.tile_pool(name="sT", bufs=2) as stp,
            tc.tile_pool(name="acts", bufs=1) as hp,
        ):
            ident_bf = constp.tile([P, P], BF16)
            make_identity(nc, ident_bf[:])

            b1sb = constp.tile([P, MC1], F32)
            b2sb = constp.tile([P, MC2], F32)
            b3sb = constp.tile([P, MC3], F32)
            nc.scalar.dma_start(b1sb[:], b1c[:, :])
            nc.scalar.dma_start(b2sb[:], b2c[:, :])
            nc.scalar.dma_start(b3sb[:], b3c[:, :])

            w1sb = wp.tile([P, KCB * MC1 * P], BF16)
            w2sb = wp.tile([P, KC2 * MC2 * P], BF16)
            w3sb = wp.tile([P, KC3 * MC3 * P], BF16)
            w1s_sb = wp.tile([P, 4, H1], BF16)
            w1st_sb = wp.tile([P, H1], BF16)
            for k in range(4):  # split so the A1 phase (k=0,1) starts ASAP
                nc.gpsimd.dma_start(w1s_sb[:, k, :], w1s[:, k, :])
            nc.gpsimd.dma_start(w1st_sb[:], w1st[:, :])
            nc.gpsimd.dma_start(w1sb[:], w1c[:, :])
            nc.gpsimd.dma_start(w2sb[:], w2c[:, :])
            nc.gpsimd.dma_start(w3sb[:], w3c[:, :])

            idx1sb = idxp.tile([P, n_tiles * SUB], I32)
            idx2sb = idxp.tile([P, n_tiles * SUB], I32)
            g2bsb = idxp.tile([P, n_tiles * SUB], I32)
            nc.sync.dma_start(idx1sb[:], idx1[:, :])
            nc.sync.dma_start(idx2sb[:], idx2[:, :])
            nc.sync.dma_start(g2bsb[:], g2b[:, :])

            PREF = 3

            def issue_g1(t, a1g, asg):
                for j in range(SUB):
                    cj = t * SUB + j
                    nc.gpsimd.indirect_dma_start(
                        out=a1g[:, j, :], out_offset=None, in_=A1d[:],
                        in_offset=bass.IndirectOffsetOnAxis(
                            ap=idx1sb[:, cj:cj + 1], axis=0),
                    )
                    nc.gpsimd.indirect_dma_start(
                        out=asg[:, j, :], out_offset=None, in_=Astd[:],
                        in_offset=bass.IndirectOffsetOnAxis(
                            ap=g2bsb[:, cj:cj + 1], axis=0),
                    )

            def issue_g2(t, a2g):
                for j in range(SUB):
                    cj = t * SUB + j
                    nc.gpsimd.indirect_dma_start(
                        out=a2g[:, j, :], out_offset=None, in_=A2d[:],
                        in_offset=bass.IndirectOffsetOnAxis(
                            ap=idx2sb[:, cj:cj + 1], axis=0),
                    )

            # ================= precompute phase =================
            with (
                tc.tile_pool(name="pstage", bufs=6) as pstage,
                tc.tile_pool(name="aout", bufs=6) as aoutp,
                tc.tile_pool(name="psA", bufs=4, space="PSUM") as psA,
            ):
                def proj_block(src_dram, dst_dram, g, k0, k1):
                    """Project 512 sites (4 chunks of 128) of src_dram
                    (feature-major [256, *]) through w1s k-chunks k0/k1
                    into rows g*512.. of dst_dram."""
                    w_rhs_k0 = lambda a, b: w1s_sb[:, k0, a:b]
                    w_rhs_k1 = lambda a, b: w1s_sb[:, k1, a:b]
                    st0 = pstage.tile([P, 4 * P], BF16, tag="st0")
                    st1 = pstage.tile([P, 4 * P], BF16, tag="st1")
                    lo = g * 4 * P
                    nc.sync.dma_start(st0[:], src_dram[0:P, lo:lo + 4 * P])
                    nc.sync.dma_start(st1[:], src_dram[P:2 * P, lo:lo + 4 * P])
                    for i in range(4):
                        ps = psA.tile([P, H1], F32, tag="psA")
                        sl = slice(i * P, (i + 1) * P)
                        nc.tensor.matmul(ps[:, 0:512], st0[:, sl],
                                         w_rhs_k0(0, 512),
                                         start=True, stop=False)
                        nc.tensor.matmul(ps[:, 512:1024], st0[:, sl],
                                         w_rhs_k0(512, 1024),
                                         start=True, stop=False)
                        nc.tensor.matmul(ps[:, 0:512], st1[:, sl],
                                         w_rhs_k1(0, 512),
                                         start=False, stop=True)
                        nc.tensor.matmul(ps[:, 512:1024], st1[:, sl],
                                         w_rhs_k1(512, 1024),
                                         start=False, stop=True)
                        ao = aoutp.tile([P, H1], BF16, tag="ao")
                        nc.vector.tensor_copy(ao[:], ps[:])
                        r = lo + i * P
                        nc.scalar.dma_start(dst_dram[r:r + P, :], ao[:])

                # ---- A1 (local idx1 range) through W1 rows 0:256 (k=0,1)
                for g in range(nloc // 4):
                    proj_block(sitesTloc, A1d, g, 0, 1)

                # ---- Ast (graph states) through W1 rows 768:896
                stt = pstage.tile([P, N_GRAPHS], BF16, tag="stt")
                nc.sync.dma_start(stt[:], statesT[:, :])
                for q in range(N_GRAPHS // P):
                    ps = psA.tile([P, H1], F32, tag="psA")
                    nc.tensor.matmul(ps[:, 0:512], stt[:, q * P:(q + 1) * P],
                                     w1st_sb[:, 0:512], start=True, stop=True)
                    nc.tensor.matmul(ps[:, 512:1024], stt[:, q * P:(q + 1) * P],
                                     w1st_sb[:, 512:1024], start=True, stop=True)
                    ao = aoutp.tile([P, H1], BF16, tag="ao")
                    nc.vector.tensor_copy(ao[:], ps[:])
                    nc.scalar.dma_start(Astd[q * P:(q + 1) * P, :], ao[:])

                # prefetch A1/Ast gather rows for the first tiles; these
                # only depend on the (small) A1/Ast tables and overlap the
                # A2 projection phase below. a2g gathers would block the
                # gpsimd queue until A2 is fully written, so they are
                # issued per-tile in the main loop.
                pre_g = {}
                for t in range(min(PREF, n_tiles)):
                    a1g = g1p.tile([P, SUB, H1], BF16, tag="a1g")
                    asg = g1p.tile([P, SUB, H1], BF16, tag="asg")
                    issue_g1(t, a1g, asg)
                    pre_g[t] = (a1g, asg)

                # ---- A2 (deduped idx2 sites) through W1 rows 256:512 (k=2,3)
                for g in range(nch2g):
                    proj_block(sitesT2, A2d, g, 2, 3)

            # NOTE: no barrier needed - the tile framework tracks DRAM RAW
            # deps (gathers wait on the A-table writes automatically), and
            # omitting it lets A1/Ast gathers overlap the A2 precompute.

            # ================= main loop =================
            with (
                tc.tile_pool(name="psmm", bufs=6, space="PSUM") as psmm,
                tc.tile_pool(name="psx", bufs=2, space="PSUM") as psx,
            ):
                for t in range(n_tiles):
                    # ---- gather projection rows: [P, SUB, H1] bf16
                    if t in pre_g:
                        a1g, asg = pre_g.pop(t)
                    else:
                        a1g = g1p.tile([P, SUB, H1], BF16, tag="a1g")
                        asg = g1p.tile([P, SUB, H1], BF16, tag="asg")
                        issue_g1(t, a1g, asg)
                    a2g = g2p.tile([P, SUB, H1], BF16, tag="a2g")
                    issue_g2(t, a2g)
                    # ---- sum the three projections (still bond-major)
                    s01 = s01p.tile([P, SUB, H1], BF16, tag="s01")
                    ssum = ssump.tile([P, SUB, H1], BF16, tag="ssum")
                    nc.vector.tensor_add(s01[:], a1g[:], a2g[:])
                    nc.vector.tensor_add(ssum[:], s01[:], asg[:])

                    # bonds arrive pre-transposed from the host: cast-DMA chunks
                    xb = []
                    for c in range(KCB):
                        xsb = xp.tile([P, T], BF16, tag=f"xTb{c}")
                        nc.sync.dma_start(
                            xsb[:], bondsT[c * P:(c + 1) * P, t * T:(t + 1) * T])
                        xb.append(xsb)

                    # ---- layer 1: bond-block matmul + transposed projection sum
                    h1T = []
                    for m in range(MC1):
                        ps = psmm.tile([P, T], F32, tag="psmm")
                        for k in range(KCB):
                            nc.tensor.matmul(
                                ps[:],
                                w1sb[:, (k * MC1 + m) * P:(k * MC1 + m + 1) * P],
                                xb[k][:],
                                start=(k == 0), stop=(k == KCB - 1),
                            )
                        pst = psx.tile([P, T], BF16, tag="psx")
                        for j in range(SUB):
                            nc.tensor.transpose(
                                pst[:, j * P:(j + 1) * P],
                                ssum[:, j, m * P:(m + 1) * P],
                                ident_bf[:],
                            )
                        sT = stp.tile([P, T], BF16, tag="sT")
                        nc.vector.tensor_copy(sT[:], pst[:])
                        pre = stp.tile([P, T], BF16, tag="pre")
                        nc.vector.tensor_add(pre[:], ps[:], sT[:])
                        hsb = hp.tile([P, T], BF16, tag=f"h1T{m}")
                        nc.scalar.activation(
                            hsb[:], pre[:], mybir.ActivationFunctionType.Relu,
                            bias=b1sb[:, m:m + 1],
                        )
                        h1T.append(hsb)

                    # ---- layer 2
                    h2T = []
                    for m in range(MC2):
                        ps = psmm.tile([P, T], F32, tag="psmm")
                        for k in range(KC2):
                            nc.tensor.matmul(
                                ps[:],
                                w2sb[:, (k * MC2 + m) * P:(k * MC2 + m + 1) * P],
                                h1T[k][:],
                                start=(k == 0), stop=(k == KC2 - 1),
                            )
                        hsb = hp.tile([P, T], BF16, tag=f"h2T{m}")
                        nc.scalar.activation(
                            hsb[:], ps[:], mybir.ActivationFunctionType.Relu,
                            bias=b2sb[:, m:m + 1],
                        )
                        h2T.append(hsb)

                    # ---- layer 3
                    oT = []
                    for m in range(MC3):
                        ps = psmm.tile([P, T], F32, tag="psmm")
                        for k in range(KC3):
                            nc.tensor.matmul(
                                ps[:],
                                w3sb[:, (k * MC3 + m) * P:(k * MC3 + m + 1) * P],
                                h2T[k][:],
                                start=(k == 0), stop=(k == KC3 - 1),
                            )
                        hsb = hp.tile([P, T], F32R, tag=f"oT{m}")
                        nc.scalar.activation(
                            hsb[:], ps[:], mybir.ActivationFunctionType.Relu,
                            bias=b3sb[:, m:m + 1],
                        )
                        oT.append(hsb)

                    # ---- store transposed output; host un-transposes
                    for c in range(MC3):
                        nc.sync.dma_start(
                            outT[c * P:(c + 1) * P, t * T:(t + 1) * T],
                            oT[c][:].bitcast(F32),
                        )

    _legalize_waits(nc)
    return nc


    _legalize_waits(nc)
    return nc


def _prep_shared(W1, b1, W2, b2, W3, b3):
    W1 = np.asarray(W1, dtype=np.float32)

    def chunk_w(W, KC, MC):
        # [KC*P, MC*P] -> [P, KC*MC*P] with w[p, (k*MC+m)*P+j] = W[k*P+p, m*P+j]
        return np.ascontiguousarray(
            W.reshape(KC, P, MC, P).transpose(1, 0, 2, 3).reshape(P, KC * MC * P)
        ).astype(np.float32, copy=False)

    def chunk_b(b, MC):
        return np.ascontiguousarray(np.asarray(b).reshape(MC, P).T).astype(
            np.float32, copy=False)

    return {
        "w1s": np.ascontiguousarray(
            W1[0:512].reshape(4, P, H1).transpose(1, 0, 2)),
        "w1st": np.ascontiguousarray(W1[768:896]),
        "w1c": chunk_w(W1[512:768], KCB, MC1),
        "w2c": chunk_w(np.asarray(W2, dtype=np.float32), KC2, MC2),
        "w3c": chunk_w(np.asarray(W3, dtype=np.float32), KC3, MC3),
        "b1c": chunk_b(b1, MC1),
        "b2c": chunk_b(b2, MC2),
        "b3c": chunk_b(b3, MC3),
    }


def _wrap_idx(raw: np.ndarray) -> np.ndarray:
    # [E_core] -> [P, n_tiles*SUB] with idx[p, q] = raw[q*P + p]
    n = raw.shape[0] // P
    return np.ascontiguousarray(raw.reshape(n, P).T).astype(np.int32, copy=False)


_BUILT = {}


def _get_bass(n_tiles: int, nloc: int, nch2g: int) -> bass.Bass:
    key = (n_tiles, nloc, nch2g)
    if key not in _BUILT:
        _BUILT[key] = build_bass(n_tiles, nloc, nch2g)
    return _BUILT[key]


def prepare(sites, bonds, states, indices1, indices2, graph_to_bonds,
            W1, b1, W2, b2, W3, b3):
    """Shard + reformat full inputs. Returns (nc, in_maps, perm, n_tiles)."""
    i1 = np.asarray(indices1).astype(np.int64, copy=False)
    i2 = np.asarray(indices2).astype(np.int64, copy=False)
    gb = np.asarray(graph_to_bonds).astype(np.int64, copy=False)
    bonds = np.asarray(bonds, dtype=np.float32)
    n_bonds = bonds.shape[0]
    assert n_bonds == N_BONDS

    perm = np.argsort(i1, kind="stable")
    i1s, i2s, gbs = i1[perm], i2[perm], gb[perm]
    bondsT_s = np.ascontiguousarray(bonds[perm].T)  # [256, n_bonds]

    # per-core shard boundaries in the sorted order; local site ranges
    starts = [c * E_SHARD for c in range(N_CORES)]
    los = [int(i1s[s]) for s in starts]
    his = [int(i1s[s + E_SHARD - 1]) for s in starts]
    sizes = [hi - lo + 1 for lo, hi in zip(los, his)]
    nloc = max(20, 4 * (-(-max(sizes) // (4 * P))))
    LSITE = nloc * P

    n_tiles = max(TILES_PER_CORE, -(-E_SHARD // T))
    e_core = n_tiles * T

    BF = ml_dtypes.bfloat16
    sitesT_bf = np.asarray(sites, dtype=np.float32).T.astype(BF)  # [256, N_SITES]
    statesT_bf = np.ascontiguousarray(
        np.asarray(states, dtype=np.float32).T).astype(BF)
    bondsT_bf = bondsT_s.astype(BF)

    # dedup idx2 per core so A2 only covers referenced sites
    refs = [np.unique(i2s[starts[c]:starts[c] + E_SHARD]) for c in range(N_CORES)]
    nch2g = max(28, max(-(-len(r) // (4 * P)) for r in refs))
    LS2 = nch2g * 4 * P

    shared = _prep_shared(W1, b1, W2, b2, W3, b3)
    in_maps = []
    for c in range(N_CORES):
        lo, hi = los[c], his[c]
        sl = slice(starts[c], starts[c] + E_SHARD)
        stl = np.zeros((SITE_LEN, LSITE), dtype=BF)
        avail = min(LSITE, N_SITES - lo)
        stl[:, :avail] = sitesT_bf[:, lo:lo + avail]

        st2 = np.zeros((SITE_LEN, LS2), dtype=BF)
        st2[:, :len(refs[c])] = sitesT_bf[:, refs[c]]

        i1_loc = np.zeros(e_core, dtype=np.int64)
        i1_loc[:E_SHARD] = i1s[sl] - lo
        i2_loc = np.zeros(e_core, dtype=np.int64)
        i2_loc[:E_SHARD] = np.searchsorted(refs[c], i2s[sl])
        gb_pad = np.zeros(e_core, dtype=np.int64)
        gb_pad[:E_SHARD] = gbs[sl]
        bT = np.zeros((BOND_LEN, e_core), dtype=BF)
        bT[:, :E_SHARD] = bondsT_bf[:, sl]

        m = {
            "sitesT2": st2,
            "sitesTloc": stl,
            "statesT": statesT_bf,
            "bondsT": bT,
            "idx1": _wrap_idx(i1_loc),
            "idx2": _wrap_idx(i2_loc),
            "g2b": _wrap_idx(gb_pad),
        }
        m.update(shared)
        in_maps.append(m)

    nc = _get_bass(n_tiles, nloc, nch2g)
    return nc, in_maps, perm, n_tiles


def kernel(sites, bonds, states, indices1, indices2, graph_to_bonds,
           W1, b1, W2, b2, W3, b3):
    nc, in_maps, perm, n_tiles = prepare(
        sites, bonds, states, indices1, indices2, graph_to_bonds,
        W1, b1, W2, b2, W3, b3)
    res = run_bass_kernel_spmd(nc, in_maps, core_ids=list(range(N_CORES)))
    out = np.empty((N_BONDS, OUT_DIM), dtype=np.float32)
    for c in range(N_CORES):
        sl = slice(c * E_SHARD, (c + 1) * E_SHARD)
        out[perm[sl]] = res.results[c]["outT"][:, :E_SHARD].T
    return out


